# revision 1
# baseline (speedup 1.0000x reference)
"""Trainium2 Bass kernel for nn_Attention_55499567399068.

Episode-attention block: per (batch, nway) pair of [64, 512] blocks:
  q/k/v linear projections -> scaled dot-product attention over nshot ->
  reduce_att MLP producing per-row weights -> weighted sum of context rows.

Sharding: pure data parallel over batch across 8 NeuronCores (32 episodes each).
Per core: 256 independent (b, n) pairs, processed in 32 superblocks of 8 pairs.

Math restructuring (exact, up to fp precision):
  - 1/sqrt(d) folded into Wq, bq on host.
  - Softmax without max-subtraction (scores are O(1)): E = exp(S) on ScalarE
    with the row-sum Z as the same instruction's accum_out; A = E * (1/Z) is a
    cheap per-partition scale in E-natural layout.
       hid = Wr1 @ C^T; leaky+br1-bias fused into one ScalarE op
       w = hid^T @ Wr2 + br2;  g = A^T @ w;  out^T[h] = sum_k v[k,h] * g[k]
  - bq, bk added via per-partition activation bias on the PSUM->SBUF copy
    (q^T/k^T layouts have h on partitions); bv added via a broadcast-tile
    tensor_tensor add on the v copy (v natural layout has h on free).

Layout strategy: inputs are loaded naturally ([row, d]), cast to bf16, and
transposed to [d, row] with the DMA xbar (2-byte transpose engine) so the
d-contraction projections run at full PE rate.
"""

import sys

sys.path.insert(0, "/opt/trn_rl_repo")

import ml_dtypes
import numpy as np

import concourse.bass as bass
import concourse.tile as tile
from concourse import bacc, mybir
from concourse.bass_utils import run_bass_kernel_spmd

F32 = mybir.dt.float32
BF16 = mybir.dt.bfloat16
BF16_NP = ml_dtypes.bfloat16

BS, NWAY, NSHOT, D = 256, 8, 64, 512
NCORES = 8
BS_SH = BS // NCORES  # 32 episodes per core
NPAIR = BS_SH * NWAY  # 256 pairs per core
SUPER = 8  # pairs per superblock
NSB = NPAIR // SUPER  # 32 superblocks
ROWS_SB = SUPER * NSHOT  # 512 rows per superblock
LEAK = 0.01
AT = mybir.ActivationFunctionType
ALU = mybir.AluOpType

BR2_VAL = [0.0]  # captured at build time as an immediate
SUB = {"act3d": True, "br1grp": True, "widelrelu": True, "zrowmm": True, "rank1": True}


def build_nc(repeat=1, cast_dma=True, n_sb=NSB, lrelu=True, widehid=False, xbar3d=False):
    nc = bacc.Bacc("TRN2", target_bir_lowering=False)

    xq = nc.dram_tensor("xq", [NPAIR * NSHOT, D], F32, kind="ExternalInput")
    xk = nc.dram_tensor("xk", [NPAIR * NSHOT, D], F32, kind="ExternalInput")
    xv = nc.dram_tensor("xv", [NPAIR * NSHOT, D], F32, kind="ExternalInput")
    wqT_d = nc.dram_tensor("wqT", [D, D], BF16, kind="ExternalInput")  # [d, h]
    wkT_d = nc.dram_tensor("wkT", [D, D], BF16, kind="ExternalInput")
    wvT_d = nc.dram_tensor("wvT", [D, D], BF16, kind="ExternalInput")
    wr1T_d = nc.dram_tensor("wr1T", [D, 64], BF16, kind="ExternalInput")  # [h, m]
    wr2T_d = nc.dram_tensor("wr2T", [64, 1], BF16, kind="ExternalInput")  # [m, 1]
    br1b_d = nc.dram_tensor("br1b", [128, 64], BF16, kind="ExternalInput")
    bvb_d = nc.dram_tensor("bvb", [128, D], BF16, kind="ExternalInput")
    bq_d = nc.dram_tensor("bq", [128, 4], F32, kind="ExternalInput")
    bk_d = nc.dram_tensor("bk", [128, 4], F32, kind="ExternalInput")
    br1c_d = nc.dram_tensor("br1c", [64, 1], F32, kind="ExternalInput")
    out_d = nc.dram_tensor("out", [NPAIR, D], F32, kind="ExternalOutput")

    with tile.TileContext(nc) as tc:
        import contextlib

        ctx = contextlib.ExitStack()
        with ctx:
            const_pool = ctx.enter_context(tc.tile_pool(name="const", bufs=1))
            ld_pool = ctx.enter_context(tc.tile_pool(name="loads", bufs=3))
            xt_pool = ctx.enter_context(tc.tile_pool(name="xt", bufs=2))
            proj_pool = ctx.enter_context(tc.tile_pool(name="projs", bufs=2))
            mid_pool = ctx.enter_context(tc.tile_pool(name="mid", bufs=2))
            out_pool = ctx.enter_context(tc.tile_pool(name="outs", bufs=2))
            psA = ctx.enter_context(tc.tile_pool(name="psA", bufs=2, space="PSUM"))
            psS = ctx.enter_context(tc.tile_pool(name="psS", bufs=3, space="PSUM"))
            psC = ctx.enter_context(tc.tile_pool(name="psC", bufs=2, space="PSUM"))
            psB = ctx.enter_context(tc.tile_pool(name="psB", bufs=1, space="PSUM"))

            wqT = const_pool.tile([128, 4 * D], BF16, tag="wqT")
            wkT = const_pool.tile([128, 4 * D], BF16, tag="wkT")
            wvT = const_pool.tile([128, 4 * D], BF16, tag="wvT")
            wr1T = const_pool.tile([128, 4 * 64], BF16, tag="wr1T")
            wr2T = const_pool.tile([64, 1], BF16, tag="wr2T")
            br1b = const_pool.tile([128, 64], BF16, tag="br1b")
            bvb = const_pool.tile([128, D], BF16, tag="bvb")
            bqs = const_pool.tile([128, 4], F32, tag="bqs")
            bks = const_pool.tile([128, 4], F32, tag="bks")
            br1c = const_pool.tile([64, 1], F32, tag="br1c")

            def load_consts():
                nc.sync.dma_start(
                    wqT[:].rearrange("p (dc h) -> p dc h", dc=4),
                    wqT_d[:, :].rearrange("(dc p) h -> p dc h", p=128),
                )
                nc.sync.dma_start(
                    wkT[:].rearrange("p (dc h) -> p dc h", dc=4),
                    wkT_d[:, :].rearrange("(dc p) h -> p dc h", p=128),
                )
                nc.sync.dma_start(
                    wvT[:].rearrange("p (dc h) -> p dc h", dc=4),
                    wvT_d[:, :].rearrange("(dc p) h -> p dc h", p=128),
                )
                nc.sync.dma_start(
                    wr1T[:].rearrange("p (hc m) -> p hc m", hc=4),
                    wr1T_d[:, :].rearrange("(hc p) m -> p hc m", p=128),
                )
                nc.sync.dma_start(wr2T[:], wr2T_d[:, :])
                nc.sync.dma_start(br1b[:], br1b_d[:, :])
                nc.sync.dma_start(bvb[:], bvb_d[:, :])
                nc.sync.dma_start(bqs[:], bq_d[:, :])
                nc.sync.dma_start(bks[:], bk_d[:, :])
                nc.sync.dma_start(br1c[:], br1c_d[:, :])

            def emit_superblock(sb):
                # ---------- Phase A: load + cast + transpose ----------
                xts = {}
                for name, src in (("q", xq), ("k", xk), ("v", xv)):
                    src_ap = src[bass.ts(sb, ROWS_SB), :].rearrange(
                        "(r p) d -> p r d", p=128
                    )
                    if cast_dma:
                        xbf = ld_pool.tile([128, 4 * D], BF16, tag=f"xbf{name}")
                        nc.gpsimd.dma_start(
                            xbf[:].rearrange("p (r d) -> p r d", r=4), src_ap
                        )
                    else:
                        xf32 = ld_pool.tile([128, 4 * D], F32, tag=f"xf{name}")
                        nc.sync.dma_start(
                            xf32[:].rearrange("p (r d) -> p r d", r=4), src_ap
                        )
                        xbf = ld_pool.tile([128, 4 * D], BF16, tag=f"xbf{name}")
                        nc.vector.tensor_copy(xbf[:], xf32[:])
                    xt = xt_pool.tile([128, 4 * D], BF16, tag=f"xt{name}")
                    if xbar3d:
                        xt4 = xt[:].rearrange("p (dc rb i) -> p dc rb i", dc=4, rb=4)
                        for r in range(4):
                            # one xbar DMA transposes a [128, 512] row-block:
                            # out[p, dc, i] = in[i, dc*128+p]
                            nc.sync.dma_start(
                                xt4[:, :, r, :],
                                xbf[:, r * 512 : (r + 1) * 512],
                                transpose=True,
                            )
                    else:
                        for r in range(4):
                            for dc in range(4):
                                nc.sync.dma_start(
                                    xt[
                                        :,
                                        dc * 512 + r * 128 : dc * 512 + (r + 1) * 128,
                                    ],
                                    xbf[
                                        :,
                                        r * 512 + dc * 128 : r * 512 + (dc + 1) * 128,
                                    ],
                                    transpose=True,
                                )
                    xts[name] = xt

                # ---------- Phase B: projections ----------
                qTs = proj_pool.tile([128, 4 * 512], BF16, tag="qTs")
                kTs = proj_pool.tile([128, 4 * 512], BF16, tag="kTs")
                vS = proj_pool.tile([128, 4 * 512], BF16, tag="vS")
                for dst, w_t, x_t, bias_t in (
                    (qTs, wqT, xts["q"], bqs),
                    (kTs, wkT, xts["k"], bks),
                ):
                    for hc in range(4):
                        ps = psA.tile([128, 512], F32, tag="proj")
                        for dc in range(4):
                            nc.tensor.matmul(
                                ps[:],
                                lhsT=w_t[
                                    :, dc * 512 + hc * 128 : dc * 512 + (hc + 1) * 128
                                ],
                                rhs=x_t[:, dc * 512 : (dc + 1) * 512],
                                start=(dc == 0),
                                stop=(dc == 3),
                            )
                        nc.scalar.activation(
                            dst[:, hc * 512 : (hc + 1) * 512],
                            ps[:],
                            AT.Identity,
                            bias=bias_t[:, hc : hc + 1],
                        )
                for r in range(4):
                    ps = psA.tile([128, 512], F32, tag="proj")
                    for dc in range(4):
                        nc.tensor.matmul(
                            ps[:],
                            lhsT=xts["v"][
                                :, dc * 512 + r * 128 : dc * 512 + (r + 1) * 128
                            ],
                            rhs=wvT[:, dc * 512 : (dc + 1) * 512],
                            start=(dc == 0),
                            stop=(dc == 3),
                        )
                    nc.vector.tensor_tensor(
                        vS[:, r * 512 : (r + 1) * 512], ps[:], bvb[:], op=ALU.add
                    )

                # ---------- Phase C ----------
                bat = psB.tile([128, 512], F32, tag="batch")
                Zb = mid_pool.tile([64, 8], F32, tag="Zb")

                # C1: scores, 8 pairs as regions of one bank
                s_all = psS.tile([64, 512], F32, tag="sh")
                for pr in range(SUPER):
                    for hc in range(4):
                        nc.tensor.matmul(
                            s_all[:, pr * 64 : (pr + 1) * 64],
                            lhsT=qTs[:, hc * 512 + pr * 64 : hc * 512 + (pr + 1) * 64],
                            rhs=kTs[:, hc * 512 + pr * 64 : hc * 512 + (pr + 1) * 64],
                            start=(hc == 0),
                            stop=(hc == 3),
                        )
                # C2: E = exp(S), Z = rowsum(E)
                e2s = []
                for pp in range(4):
                    e2 = mid_pool.tile([64, 128], BF16, tag=f"e2_{pp}")
                    e2s.append(e2)
                for pr in range(SUPER):
                    pp, half = pr // 2, pr % 2
                    nc.scalar.activation(
                        e2s[pp][:, half * 64 : (half + 1) * 64],
                        s_all[:, pr * 64 : (pr + 1) * 64],
                        AT.Exp,
                        accum_out=Zb[:, pr : pr + 1],
                    )
                # C2b: normalize in place: A = E * (1/Z), per-partition scale
                rT = mid_pool.tile([64, 8], F32, tag="rT")
                nc.vector.reciprocal(rT[:], Zb[:])
                for pr in range(SUPER):
                    pp, half = pr // 2, pr % 2
                    nc.vector.tensor_scalar(
                        e2s[pp][:, half * 64 : (half + 1) * 64],
                        e2s[pp][:, half * 64 : (half + 1) * 64],
                        rT[:, pr : pr + 1],
                        None,
                        op0=ALU.mult,
                    )
                # C3: A^T via xbar
                ets = []
                for pp in range(4):
                    et = mid_pool.tile([128, 64], BF16, tag=f"et_{pp}")
                    nc.sync.dma_start(et[:], e2s[pp][:], transpose=True)
                    ets.append(et)

                ys_all = mid_pool.tile([64, 512], BF16, tag="ys_all")
                lr = AT.Lrelu if lrelu else AT.Relu
                if widehid:
                    # C4: C~^T per pair into combined [p, hc, (pr q)] tile
                    cts_all = mid_pool.tile([128, 4 * 512], BF16, tag="cts_all")
                    cts3 = cts_all[:].rearrange("p (hc prq) -> p hc prq", hc=4)
                    for pr in range(SUPER):
                        pp, half = pr // 2, pr % 2
                        ctp = psC.tile([128, 256], F32, tag="ct")
                        for hc in range(4):
                            nc.tensor.matmul(
                                ctp[:, hc * 64 : (hc + 1) * 64],
                                lhsT=vS[
                                    half * 64 : (half + 1) * 64,
                                    pp * 512 + hc * 128 : pp * 512 + (hc + 1) * 128,
                                ],
                                rhs=ets[pp][half * 64 : (half + 1) * 64, :],
                                start=True,
                                stop=True,
                            )
                        dst = cts3[:, :, pr * 64 : (pr + 1) * 64]
                        src3 = ctp[:].rearrange("p (hc q) -> p hc q", hc=4)
                        if pr % 2 == 0 or not SUB["act3d"]:
                            nc.vector.tensor_copy(dst, src3)
                        else:
                            nc.scalar.activation(dst, src3, AT.Copy)
                    # C5: hid = 4 wide matmuls (A is normalized, C is true context)
                    hid_all = psS.tile([64, 512], F32, tag="sh")
                    for hc in range(4):
                        nc.tensor.matmul(
                            hid_all[:],
                            lhsT=wr1T[:, hc * 64 : (hc + 1) * 64],
                            rhs=cts_all[:, hc * 512 : (hc + 1) * 512],
                            start=(hc == 0),
                            stop=(hc == 3),
                        )
                    # C6: leaky relu (+ br1 per-partition bias) over all pairs
                    nc.scalar.activation(
                        ys_all[:], hid_all[:], lr, bias=br1c[:], alpha=LEAK
                    )
                else:
                    # v1-style per-pair C~/hid/leaky
                    for pr in range(SUPER):
                        pp, half = pr // 2, pr % 2
                        ctp = psC.tile([128, 256], F32, tag="ct")
                        for hc in range(4):
                            nc.tensor.matmul(
                                ctp[:, hc * 64 : (hc + 1) * 64],
                                lhsT=vS[
                                    half * 64 : (half + 1) * 64,
                                    pp * 512 + hc * 128 : pp * 512 + (hc + 1) * 128,
                                ],
                                rhs=ets[pp][half * 64 : (half + 1) * 64, :],
                                start=True,
                                stop=True,
                            )
                        cts = mid_pool.tile([128, 256], BF16, tag="cts")
                        nc.vector.tensor_copy(cts[:], ctp[:])
                        hid_t = psS.tile([64, 512], F32, tag="sh")
                        hid = hid_t[:, 0:64]
                        for hc in range(4):
                            nc.tensor.matmul(
                                hid,
                                lhsT=wr1T[:, hc * 64 : (hc + 1) * 64],
                                rhs=cts[:, hc * 64 : (hc + 1) * 64],
                                start=(hc == 0),
                                stop=(hc == 3),
                            )
                        nc.scalar.activation(
                            ys_all[:, pr * 64 : (pr + 1) * 64], hid, lr,
                            bias=br1c[:], alpha=LEAK,
                        )

                # C7: w~^T columns
                for pr in range(SUPER):
                    nc.tensor.matmul(
                        bat[0:64, pr : pr + 1],
                        lhsT=ys_all[:, pr * 64 : (pr + 1) * 64],
                        rhs=wr2T[:],
                        start=True,
                        stop=True,
                    )
                # C8: w = w~ + br2 (A was normalized, so no Z scaling here)
                wh = mid_pool.tile([64, 8], BF16, tag="wh")
                nc.vector.tensor_scalar(
                    wh[:], bat[0:64, 0:8], float(BR2_VAL[0]), None, op0=ALU.add
                )
                # C9: g = E^T @ w^
                for pr in range(SUPER):
                    pp, half = pr // 2, pr % 2
                    nc.tensor.matmul(
                        bat[0:64, 8 + pr : 9 + pr],
                        lhsT=e2s[pp][:, half * 64 : (half + 1) * 64],
                        rhs=wh[:, pr : pr + 1],
                        start=True,
                        stop=True,
                    )
                gS = mid_pool.tile([128, 8], BF16, tag="gS")
                nc.vector.tensor_copy(gS[0:64, :], bat[0:64, 8:16])
                nc.vector.tensor_copy(gS[64:128, :], bat[0:64, 8:16])
                # C10: out^T chunks
                for pr in range(SUPER):
                    pp, half = pr // 2, pr % 2
                    for hc in range(4):
                        nc.tensor.matmul(
                            bat[:, 16 + pr * 4 + hc : 17 + pr * 4 + hc],
                            lhsT=vS[
                                half * 64 : (half + 1) * 64,
                                pp * 512 + hc * 128 : pp * 512 + (hc + 1) * 128,
                            ],
                            rhs=gS[half * 64 : (half + 1) * 64, pr : pr + 1],
                            start=True,
                            stop=True,
                        )
                outTs = out_pool.tile([128, 32], F32, tag="outTs")
                nc.scalar.activation(outTs[:], bat[:, 16:48], AT.Copy)
                outN = out_pool.tile([32, 128], F32, tag="outN")
                for b in range(4):
                    nc.vector.transpose(
                        outN[0:32, b * 32 : (b + 1) * 32],
                        outTs[b * 32 : (b + 1) * 32, 0:32],
                    )
                nc.sync.dma_start(
                    out_d[bass.ts(sb, SUPER), :].rearrange(
                        "pr (hc c) -> (pr hc) c", hc=4
                    ),
                    outN[:],
                )

            def body(_iv=None):
                load_consts()
                for sb in range(n_sb):
                    emit_superblock(sb)

            if repeat == 1:
                body()
            else:
                with tc.For_i(0, repeat, 1) as _iv:
                    body(_iv)

    nc.compile()
    return nc


def prep_in_maps(query, key, value, Wq, bq, Wk, bk, Wv, bv, Wr1, br1, Wr2, br2):
    """Host-side prep: shard + weight transforms. Returns in_maps list of 8 dicts."""
    s = 1.0 / np.sqrt(np.float32(D))
    wqT = (Wq * s).T.astype(BF16_NP).copy()  # [d, h]
    wkT = Wk.T.astype(BF16_NP).copy()
    wvT = Wv.T.astype(BF16_NP).copy()
    wr1T = Wr1.T.astype(BF16_NP).copy()  # [h, m]
    wr2T = Wr2.T.astype(BF16_NP).copy()  # [m, 1]
    br1b = np.tile(br1[None, :].astype(np.float32), (128, 1)).astype(BF16_NP)
    bvb = np.tile(bv[None, :].astype(np.float32), (128, 1)).astype(BF16_NP)
    bqv = (bq * s).astype(np.float32).reshape(4, 128).T.copy()  # [p, hc]
    bkv = bk.astype(np.float32).reshape(4, 128).T.copy()
    br1c = br1.astype(np.float32).reshape(64, 1).copy()
    BR2_VAL[0] = float(br2[0])

    in_maps = []
    for c in range(NCORES):
        sl = slice(c * BS_SH, (c + 1) * BS_SH)
        in_maps.append(
            {
                "xq": np.ascontiguousarray(query[sl]).reshape(NPAIR * NSHOT, D),
                "xk": np.ascontiguousarray(key[sl]).reshape(NPAIR * NSHOT, D),
                "xv": np.ascontiguousarray(value[sl]).reshape(NPAIR * NSHOT, D),
                "wqT": wqT,
                "wkT": wkT,
                "wvT": wvT,
                "wr1T": wr1T,
                "wr2T": wr2T,
                "br1b": br1b,
                "bvb": bvb,
                "bq": bqv,
                "bk": bkv,
                "br1c": br1c,
            }
        )
    return in_maps


_nc_cache = {}


def kernel(**inputs):
    in_maps = prep_in_maps(**{k: np.asarray(v) for k, v in inputs.items()})
    key = ("k", 1, BR2_VAL[0])
    if key not in _nc_cache:
        _nc_cache[key] = build_nc(repeat=1)
    nc = _nc_cache[key]
    res = run_bass_kernel_spmd(nc, in_maps, core_ids=list(range(NCORES)))
    outs = [res.results[c]["out"].reshape(BS_SH, NWAY, D) for c in range(NCORES)]
    return np.concatenate(outs, axis=0).astype(np.float32)



# revision 25
# speedup vs baseline: 6.7481x; 6.7481x over previous
"""Trainium2 Bass kernel for nn_Attention_55499567399068.

Episode-attention block, data-parallel over batch across 8 NeuronCores
(32 episodes = 256 (b, n) pairs per core, processed as 32 superblocks of
8 pairs x 64 rows).

Math restructuring (exact up to fp rounding):
  - q/k projections are never materialized. Scores only need
      softmax((Xq Wq^T + bq)(Xk Wk^T + bk)^T / sqrt(d));
    terms constant along a softmax row cancel, so with
      M = Wq^T Wk / sqrt(d),  wkb = Wk^T bq / sqrt(d)
    softmax(S) == softmax(t' Xk^T) where t' = Xq M + 1 wkb^T.
  - v projection is deferred past the attention matrix: A rows sum to 1, so
      context^T = Wv (Xv^T A^T) + bv 1^T.
    Xv is consumed in natural row-major layout (no transpose needed).
  - Final reduction: out^T = Wv (Xv^T (A^T w)) + bv (1^T w).
  - Softmax without max-subtraction (scores are O(1)): E = exp(S) on ScalarE
    with row-sum Z as accum_out; A = E * (1/Z) per-partition scale.
  - Leaky ReLU uses AT.Prelu (parametric relu, alpha=0.01), which shares an
    activation table with Exp/Identity/Copy -> no act-table reloads.

Hardware notes (found by bisecting exec-unit crashes):
  - Xq/Xk are transposed with the PE array (transpose-mode matmuls into bf16
    PSUM + DVE copy out), not the DMA xbar: the xbar costs ~625ns of HWDGE
    occupancy per tile and was the baseline bottleneck.
  - Consecutive K<128 matmuls whose operand partition offsets alternate
    (bass auto-derives PE tile_position (0,0)/(64,0)) into the same PSUM
    bank kill the exec unit. Per-pair K=64 contractions are instead built as
    K=128 block-diagonal matmuls: A^T / g tiles are zero-padded so the
    off-pair partitions contribute exact zeros, and every matmul runs at
    tile_position (0,0).

Emission is software-pipelined: superblock i+1's loads/transposes/t' are
emitted between superblock i's scores and its softmax-onward tail, so the
PE has independent work queued while the exp->normalize->A^T cross-engine
chain resolves.
"""

import sys

sys.path.insert(0, "/opt/trn_rl_repo")

import ml_dtypes
import numpy as np

import concourse.bass as bass
import concourse.tile as tile
from concourse import bacc, mybir
from concourse.bass_utils import run_bass_kernel_spmd

F32 = mybir.dt.float32
BF16 = mybir.dt.bfloat16
BF16_NP = ml_dtypes.bfloat16

BS, NWAY, NSHOT, D = 256, 8, 64, 512
NCORES = 8
BS_SH = BS // NCORES  # 32 episodes per core
NPAIR = BS_SH * NWAY  # 256 pairs per core
SUPER = 8  # pairs per superblock
NSB = NPAIR // SUPER  # 32 superblocks
ROWS_SB = SUPER * NSHOT  # 512 rows per superblock
LEAK = 0.01
AT = mybir.ActivationFunctionType
ALU = mybir.AluOpType

BR2_VAL = [0.0]  # captured at build time as an immediate
LEAKY_MODE = ["prelu"]  # "prelu" | "lrelu" | "dve"
PIPELINE = [True]


def build_nc(repeat=1, n_sb=NSB):
    nc = bacc.Bacc("TRN2", target_bir_lowering=False)

    xq_d = nc.dram_tensor("xq", [NPAIR * NSHOT, D], F32, kind="ExternalInput")
    xk_d = nc.dram_tensor("xk", [NPAIR * NSHOT, D], F32, kind="ExternalInput")
    xv_d = nc.dram_tensor("xv", [NPAIR * NSHOT, D], F32, kind="ExternalInput")
    mT_d = nc.dram_tensor("mT", [D, D], BF16, kind="ExternalInput")  # [d, m]
    wvT_d = nc.dram_tensor("wvT", [D, D], BF16, kind="ExternalInput")  # [d, h]
    wr1T_d = nc.dram_tensor("wr1T", [D, 64], BF16, kind="ExternalInput")  # [h, m]
    wr2T_d = nc.dram_tensor("wr2T", [64, 1], BF16, kind="ExternalInput")  # [m, 1]
    wkb_d = nc.dram_tensor("wkb", [128, 4], F32, kind="ExternalInput")
    bvc_d = nc.dram_tensor("bvc", [128, 4], F32, kind="ExternalInput")
    br1c_d = nc.dram_tensor("br1c", [64, 1], F32, kind="ExternalInput")
    bvr_d = nc.dram_tensor("bvr", [1, D], BF16, kind="ExternalInput")
    ones_d = nc.dram_tensor("ones", [64, 1], BF16, kind="ExternalInput")
    ident_d = nc.dram_tensor("ident", [128, 128], BF16, kind="ExternalInput")
    out_d = nc.dram_tensor("out", [NPAIR, D], F32, kind="ExternalOutput")

    with tile.TileContext(nc) as tc:
        import contextlib

        ctx = contextlib.ExitStack()
        with ctx:
            const_pool = ctx.enter_context(tc.tile_pool(name="const", bufs=1))
            ld_pool = ctx.enter_context(tc.tile_pool(name="loads", bufs=3))
            xt_pool = ctx.enter_context(tc.tile_pool(name="xt", bufs=2))
            mid_pool = ctx.enter_context(tc.tile_pool(name="mid", bufs=2))
            out_pool = ctx.enter_context(tc.tile_pool(name="outs", bufs=2))
            # PSUM: exactly 8 banks: psT 2 + psG 2 + psS 1 + psP 2 + psB 1.
            psT = ctx.enter_context(tc.tile_pool(name="psT", bufs=2, space="PSUM"))
            psG = ctx.enter_context(tc.tile_pool(name="psG", bufs=2, space="PSUM"))
            psS = ctx.enter_context(tc.tile_pool(name="psS", bufs=1, space="PSUM"))
            psP = ctx.enter_context(tc.tile_pool(name="psP", bufs=2, space="PSUM"))
            psB = ctx.enter_context(tc.tile_pool(name="psB", bufs=1, space="PSUM"))

            mT = const_pool.tile([128, 4 * D], BF16, tag="mT")
            wvT = const_pool.tile([128, 4 * D], BF16, tag="wvT")
            wr1T = const_pool.tile([128, 4 * 64], BF16, tag="wr1T")
            wr2T = const_pool.tile([64, 1], BF16, tag="wr2T")
            wkbs = const_pool.tile([128, 4], F32, tag="wkbs")
            bvs = const_pool.tile([128, 4], F32, tag="bvs")
            br1c = const_pool.tile([64, 1], F32, tag="br1c")
            bvr = const_pool.tile([1, D], BF16, tag="bvr")
            ones_c = const_pool.tile([64, 1], BF16, tag="ones")
            ident = const_pool.tile([128, 128], BF16, tag="ident")
            # Persistent zero-padded block-diagonal tiles (2 sets, sb%2):
            # only the diagonal blocks are rewritten each superblock.
            gZs = []
            for s in range(2):
                gZ_t = const_pool.tile([128, 8], BF16, tag=f"gZ{s}")
                gZs.append(gZ_t)
            etnB = []
            for s in range(2):
                row = []
                for r in range(4):
                    eB_t = const_pool.tile([128, 128], BF16, tag=f"etnB{s}_{r}")
                    row.append(eB_t)
                etnB.append(row)

            def load_consts():
                nc.sync.dma_start(
                    mT[:].rearrange("p (dc m) -> p dc m", dc=4),
                    mT_d[:, :].rearrange("(dc p) m -> p dc m", p=128),
                )
                nc.sync.dma_start(
                    wvT[:].rearrange("p (dc h) -> p dc h", dc=4),
                    wvT_d[:, :].rearrange("(dc p) h -> p dc h", p=128),
                )
                nc.sync.dma_start(
                    wr1T[:].rearrange("p (hc m) -> p hc m", hc=4),
                    wr1T_d[:, :].rearrange("(hc p) m -> p hc m", p=128),
                )
                nc.sync.dma_start(wr2T[:], wr2T_d[:, :])
                nc.sync.dma_start(wkbs[:], wkb_d[:, :])
                nc.sync.dma_start(bvs[:], bvc_d[:, :])
                nc.sync.dma_start(br1c[:], br1c_d[:, :])
                nc.sync.dma_start(bvr[:], bvr_d[:, :])
                nc.sync.dma_start(ones_c[:], ones_d[:, :])
                nc.sync.dma_start(ident[:], ident_d[:, :])
                for s in range(2):
                    for r in range(4):
                        nc.vector.memset(etnB[s][r][:], 0.0)
                    nc.vector.memset(gZs[s][:], 0.0)

            def emit_head(sb):
                """Loads + PE transposes + t' GEMM for superblock sb."""
                # A: load natural layout, fp32->bf16 cast in the DMA (SWDGE).
                nat = {}
                for name, src in (("q", xq_d), ("k", xk_d), ("v", xv_d)):
                    xbf = ld_pool.tile([128, 4 * D], BF16, tag=f"x{name}")
                    nc.gpsimd.dma_start(
                        xbf[:].rearrange("p (r d) -> p r d", r=4),
                        src[bass.ts(sb, ROWS_SB), :].rearrange(
                            "(r p) d -> p r d", p=128
                        ),
                    )
                    nat[name] = xbf

                # B: PE-transpose Xq, Xk -> [d, row].
                xqT = xt_pool.tile([128, 4 * D], BF16, tag="xqT")
                xkT = xt_pool.tile([128, 4 * D], BF16, tag="xkT")
                for name, xT in (("q", xqT), ("k", xkT)):
                    for dcp in range(2):  # dc pairs (0,1) and (2,3)
                        bank = psT.tile([128, 1024], BF16, tag="xt")
                        for dch in range(2):
                            dc = dcp * 2 + dch
                            for r in range(4):
                                nc.tensor.transpose(
                                    bank[
                                        :,
                                        dch * 512 + r * 128 : dch * 512 + (r + 1) * 128,
                                    ],
                                    nat[name][
                                        :,
                                        r * 512 + dc * 128 : r * 512 + (dc + 1) * 128,
                                    ],
                                    ident[:, :],
                                )
                        nc.vector.tensor_copy(
                            xT[:, dcp * 1024 : (dcp + 1) * 1024], bank[:]
                        )

                # C: t' = Xq M (+ wkb as per-partition bias on the PSUM copy).
                tT = mid_pool.tile([128, 4 * D], BF16, tag="tT")
                for mc in range(4):
                    acc = psG.tile([128, 512], F32, tag="acc")
                    for dc in range(4):
                        nc.tensor.matmul(
                            acc[:],
                            lhsT=mT[:, dc * 512 + mc * 128 : dc * 512 + (mc + 1) * 128],
                            rhs=xqT[:, dc * 512 : (dc + 1) * 512],
                            start=(dc == 0),
                            stop=(dc == 3),
                        )
                    nc.scalar.activation(
                        tT[:, mc * 512 : (mc + 1) * 512],
                        acc[:],
                        AT.Identity,
                        bias=wkbs[:, mc : mc + 1],
                    )
                return {"nat": nat, "xkT": xkT, "tT": tT}

            def emit_scores(sb, st):
                # D: S = t' Xk^T, per pair, accumulated over m chunks.
                s_all = psS.tile([64, 512], F32, tag="sh")
                for pr in range(SUPER):
                    for mc in range(4):
                        nc.tensor.matmul(
                            s_all[:, pr * 64 : (pr + 1) * 64],
                            lhsT=st["tT"][
                                :, mc * 512 + pr * 64 : mc * 512 + (pr + 1) * 64
                            ],
                            rhs=st["xkT"][
                                :, mc * 512 + pr * 64 : mc * 512 + (pr + 1) * 64
                            ],
                            start=(mc == 0),
                            stop=(mc == 3),
                        )
                st["s_all"] = s_all

            def emit_tail(sb, st):
                nat, s_all = st["nat"], st["s_all"]

                # E: E = exp(S) with row-sum accum; normalize A = E/Z in place.
                Zb = mid_pool.tile([64, 8], F32, tag="Zb")
                e2s = []
                for pp in range(4):
                    e2 = mid_pool.tile([64, 128], BF16, tag=f"e2_{pp}")
                    e2s.append(e2)
                for pr in range(SUPER):
                    pp, half = pr // 2, pr % 2
                    nc.scalar.activation(
                        e2s[pp][:, half * 64 : (half + 1) * 64],
                        s_all[:, pr * 64 : (pr + 1) * 64],
                        AT.Exp,
                        accum_out=Zb[:, pr : pr + 1],
                    )
                rT = mid_pool.tile([64, 8], F32, tag="rT")
                nc.vector.reciprocal(rT[:], Zb[:])
                for pr in range(SUPER):
                    pp, half = pr // 2, pr % 2
                    nc.vector.tensor_scalar(
                        e2s[pp][:, half * 64 : (half + 1) * 64],
                        e2s[pp][:, half * 64 : (half + 1) * 64],
                        rT[:, pr : pr + 1],
                        None,
                        op0=ALU.mult,
                    )

                # F: A^T via PE transpose into block-diagonal tiles.
                eB = etnB[sb % 2]
                etn_ps = psT.tile([128, 1024], BF16, tag="xt")
                for pp in range(4):
                    nc.tensor.transpose(
                        etn_ps[:, pp * 64 : (pp + 1) * 64],
                        e2s[pp][:],
                        ident[0:64, 0:64],
                    )
                for pp in range(4):
                    nc.vector.tensor_copy(
                        eB[pp][0:64, 0:64], etn_ps[0:64, pp * 64 : (pp + 1) * 64]
                    )
                    nc.vector.tensor_copy(
                        eB[pp][64:128, 64:128],
                        etn_ps[64:128, pp * 64 : (pp + 1) * 64],
                    )

                # G: P^T = Xv^T A^T as K=128 block-diagonal matmuls.
                ptS = mid_pool.tile([128, 4 * D], BF16, tag="ptS")
                for dc in range(4):
                    ptb = psP.tile([128, 512], F32, tag="pt")
                    for r in range(4):
                        nc.tensor.matmul(
                            ptb[:, r * 128 : (r + 1) * 128],
                            lhsT=nat["v"][
                                :, r * 512 + dc * 128 : r * 512 + (dc + 1) * 128
                            ],
                            rhs=eB[r][:],
                            start=True,
                            stop=True,
                        )
                    nc.vector.tensor_copy(ptS[:, dc * 512 : (dc + 1) * 512], ptb[:])

                # H: ct = Wv P^T (+ bv as per-partition bias on the PSUM copy).
                ctS = mid_pool.tile([128, 4 * D], BF16, tag="ctS")
                for hc in range(4):
                    acc = psG.tile([128, 512], F32, tag="acc")
                    for dc in range(4):
                        nc.tensor.matmul(
                            acc[:],
                            lhsT=wvT[
                                :, dc * 512 + hc * 128 : dc * 512 + (hc + 1) * 128
                            ],
                            rhs=ptS[:, dc * 512 : (dc + 1) * 512],
                            start=(dc == 0),
                            stop=(dc == 3),
                        )
                    nc.scalar.activation(
                        ctS[:, hc * 512 : (hc + 1) * 512],
                        acc[:],
                        AT.Identity,
                        bias=bvs[:, hc : hc + 1],
                    )

                # I: hid = Wr1 ct; leaky relu (+br1) -> ys.
                hid_t = psP.tile([128, 512], F32, tag="pt")
                hid = hid_t[0:64, :]
                for hc in range(4):
                    nc.tensor.matmul(
                        hid,
                        lhsT=wr1T[:, hc * 64 : (hc + 1) * 64],
                        rhs=ctS[:, hc * 512 : (hc + 1) * 512],
                        start=(hc == 0),
                        stop=(hc == 3),
                    )
                ys = mid_pool.tile([64, 512], BF16, tag="ys")
                if LEAKY_MODE[0] == "dve":
                    h1 = mid_pool.tile([64, 512], F32, tag="h1")
                    h2 = mid_pool.tile([64, 512], BF16, tag="h2")
                    nc.vector.tensor_scalar(h1[:], hid, br1c[:], None, op0=ALU.add)
                    nc.vector.tensor_scalar(h2[:], h1[:], LEAK, None, op0=ALU.mult)
                    nc.vector.tensor_tensor(ys[:], h1[:], h2[:], op=ALU.max)
                else:
                    nc.scalar.activation(
                        ys[:],
                        hid,
                        AT.Prelu if LEAKY_MODE[0] == "prelu" else AT.Lrelu,
                        bias=br1c[:],
                        alpha=LEAK,
                    )

                # J: w~ = ys^T wr2; w = w~ + br2; sw = 1^T w.
                bat = psB.tile([128, 512], F32, tag="bat")
                for pr in range(SUPER):
                    nc.tensor.matmul(
                        bat[0:64, pr : pr + 1],
                        lhsT=ys[:, pr * 64 : (pr + 1) * 64],
                        rhs=wr2T[:],
                        start=True,
                        stop=True,
                    )
                wh = mid_pool.tile([64, 8], BF16, tag="wh")
                nc.vector.tensor_scalar(
                    wh[:], bat[0:64, 0:8], float(BR2_VAL[0]), None, op0=ALU.add
                )
                nc.tensor.matmul(
                    bat[0:1, 80:88], lhsT=ones_c[:], rhs=wh[:], start=True, stop=True
                )
                sws = mid_pool.tile([1, 8], BF16, tag="sws")
                nc.vector.tensor_copy(sws[:], bat[0:1, 80:88])

                # K: g = A^T w (junk halves masked out via zero-padded gZ).
                for pr in range(SUPER):
                    pp = pr // 2
                    nc.tensor.matmul(
                        bat[:, 8 + pr : 9 + pr],
                        lhsT=e2s[pp][:],
                        rhs=wh[:, pr : pr + 1],
                        start=True,
                        stop=True,
                    )
                gZ = gZs[sb % 2]
                nc.vector.tensor_copy(
                    gZ[0:64, :].rearrange("p (q t) -> p q t", t=2)[:, :, 0:1],
                    bat[0:64, 8:16].rearrange("p (q t) -> p q t", t=2)[:, :, 0:1],
                )
                nc.vector.tensor_copy(
                    gZ[64:128, :].rearrange("p (q t) -> p q t", t=2)[:, :, 1:2],
                    bat[64:128, 8:16].rearrange("p (q t) -> p q t", t=2)[:, :, 1:2],
                )

                # L: y = Xv^T g as K=128 block-diagonal matmuls.
                for dc in range(4):
                    for r in range(4):
                        nc.tensor.matmul(
                            bat[:, 16 + dc * 8 + 2 * r : 18 + dc * 8 + 2 * r],
                            lhsT=nat["v"][
                                :, r * 512 + dc * 128 : r * 512 + (dc + 1) * 128
                            ],
                            rhs=gZ[:, 2 * r : 2 * r + 2],
                            start=True,
                            stop=True,
                        )
                ysb = mid_pool.tile([128, 32], BF16, tag="ysb")
                nc.vector.tensor_copy(ysb[:], bat[:, 16:48])

                # M: out^T = Wv y + bv sw (rank-1 K=1 matmul into the accum).
                for hc in range(4):
                    for dc in range(4):
                        nc.tensor.matmul(
                            bat[:, 48 + hc * 8 : 56 + hc * 8],
                            lhsT=wvT[
                                :, dc * 512 + hc * 128 : dc * 512 + (hc + 1) * 128
                            ],
                            rhs=ysb[:, dc * 8 : (dc + 1) * 8],
                            start=(dc == 0),
                            stop=False,
                        )
                    nc.tensor.matmul(
                        bat[:, 48 + hc * 8 : 56 + hc * 8],
                        lhsT=bvr[0:1, hc * 128 : (hc + 1) * 128],
                        rhs=sws[:],
                        start=False,
                        stop=True,
                    )
                outTs = out_pool.tile([128, 32], F32, tag="outTs")
                # bat cols are (hc, pr); store outTs cols as (pr, hc) so the
                # final DMA's partition group (pr hc) is adjacent.
                nc.scalar.activation(
                    outTs[:].rearrange("p (pr hc) -> p hc pr", hc=4),
                    bat[:, 48:80].rearrange("p (hc pr) -> p hc pr", hc=4),
                    AT.Copy,
                )
                outN = out_pool.tile([32, 128], F32, tag="outN")
                for b in range(4):
                    nc.vector.transpose(
                        outN[0:32, b * 32 : (b + 1) * 32],
                        outTs[b * 32 : (b + 1) * 32, 0:32],
                    )
                nc.sync.dma_start(
                    out_d[bass.ts(sb, SUPER), :].rearrange(
                        "pr (hc c) -> (pr hc) c", hc=4
                    ),
                    outN[:],
                )

            def body(_iv=None):
                if PIPELINE[0]:
                    st = emit_head(0)
                    for sb in range(n_sb):
                        emit_scores(sb, st)
                        nxt = emit_head(sb + 1) if sb + 1 < n_sb else None
                        emit_tail(sb, st)
                        st = nxt
                else:
                    for sb in range(n_sb):
                        st = emit_head(sb)
                        emit_scores(sb, st)
                        emit_tail(sb, st)

            load_consts()
            if repeat == 1:
                body()
            else:
                with tc.For_i(0, repeat, 1) as _iv:
                    body(_iv)

    nc.compile()
    return nc


def prep_in_maps(query, key, value, Wq, bq, Wk, bk, Wv, bv, Wr1, br1, Wr2, br2):
    """Host-side prep: shard + weight transforms. Returns in_maps list of 8 dicts."""
    s = 1.0 / np.sqrt(np.float32(D))
    M = (Wq.astype(np.float64).T @ Wk.astype(np.float64) * s).astype(np.float32)
    wkb = (Wk.astype(np.float64).T @ bq.astype(np.float64) * s).astype(np.float32)
    mT = M.astype(BF16_NP).copy()  # [Xq-feature (contracted with Xq), m]
    wvT = Wv.T.astype(BF16_NP).copy()  # [d, h]
    wr1T = Wr1.T.astype(BF16_NP).copy()  # [h, m]
    wr2T = Wr2.T.astype(BF16_NP).copy()  # [m, 1]
    wkbc = wkb.reshape(4, 128).T.copy()  # [p, mc]
    bvc = bv.astype(np.float32).reshape(4, 128).T.copy()  # [p, hc]
    br1c = br1.astype(np.float32).reshape(64, 1).copy()
    bvr = bv.astype(np.float32).reshape(1, D).astype(BF16_NP).copy()
    ones = np.ones((64, 1), dtype=BF16_NP)
    ident = np.eye(128, dtype=BF16_NP)
    BR2_VAL[0] = float(br2[0])

    in_maps = []
    for c in range(NCORES):
        sl = slice(c * BS_SH, (c + 1) * BS_SH)
        in_maps.append(
            {
                "xq": np.ascontiguousarray(query[sl]).reshape(NPAIR * NSHOT, D),
                "xk": np.ascontiguousarray(key[sl]).reshape(NPAIR * NSHOT, D),
                "xv": np.ascontiguousarray(value[sl]).reshape(NPAIR * NSHOT, D),
                "mT": mT,
                "wvT": wvT,
                "wr1T": wr1T,
                "wr2T": wr2T,
                "wkb": wkbc,
                "bvc": bvc,
                "br1c": br1c,
                "bvr": bvr,
                "ones": ones,
                "ident": ident,
            }
        )
    return in_maps


_nc_cache = {}


def kernel(**inputs):
    in_maps = prep_in_maps(**{k: np.asarray(v) for k, v in inputs.items()})
    key = ("k", 1, BR2_VAL[0])
    if key not in _nc_cache:
        _nc_cache[key] = build_nc(repeat=1)
    nc = _nc_cache[key]
    res = run_bass_kernel_spmd(nc, in_maps, core_ids=list(range(NCORES)))
    outs = [res.results[c]["out"].reshape(BS_SH, NWAY, D) for c in range(NCORES)]
    return np.concatenate(outs, axis=0).astype(np.float32)
